# revision 11
# baseline (speedup 1.0000x reference)
"""AttCML distributed Bass kernel for 8 TRN2 NeuronCores — TensorEngine version.

Sharding: data-parallel over batch (16384 samples assigned freely to cores).

Both attention contractions run on the (otherwise idle) PE array instead of
DVE, which was the baseline bottleneck at ~88% busy:

  - samples are packed into "quads": 128 partition rows = s samples x c pref
    slots, geometry pools c in {16, 32, 42, 64} chosen so n+1 <= c.
  - stage A (scores):  per quad  w[(k,j), (k',t)] = prefT^T @ tgt   with the
    fp8 d-major pref as FWL stationary and 2s target columns streaming; Q
    quads fill 128 PSUM columns per group, 4 groups share one bank
    ("superblock" [128, 512]) so softmax runs as single wide DVE/ACT ops.
  - softmax: DVE adds a per-group block-mask bias (-30 off-block), ACT exps
    to fp8 (off-block underflows to exact 0, killing cross-sample terms).
  - denominator: a matmul with an all-ones [128,128] stationary gives S
    broadcast across partitions; a 1-partition matmul accumulates the
    host-side pad-count correction; DVE reciprocal.
  - stage C (pooling): per quad  r^T[d, (k,t)] = prefQ^T @ e  with the fp8
    slot-major pref as FWL stationary and masked e columns streaming.
  - distances: r^T * (1/S)  (DVE) + diff0^T (GpSimd; host-precomputed
    u - tgt), squared on ACT, summed over d by a ones-column matmul,
    [1, 512] rows copied out per superblock.

Pad slots are zero rows: they add exp(0)=1 to the raw denominator (fixed by
the padc matmul) and 0 to the pooled vector — exact reference semantics.
"""

import numpy as np
from contextlib import ExitStack

try:
    import concourse  # noqa: F401
except ImportError:  # pragma: no cover
    import sys

    for _p in ("/opt/trn_rl_repo", "/root/.axon_site/_ro/trn_rl_repo"):
        if _p not in sys.path:
            sys.path.insert(0, _p)

import ml_dtypes
import concourse.bacc as bacc
import concourse.bass as bass
import concourse.tile as tile
from concourse import mybir
from concourse.bass_utils import run_bass_kernel_spmd

F32 = mybir.dt.float32
BF16 = mybir.dt.bfloat16
FP8 = mybir.dt.float8e3  # e3m4
ALU = mybir.AluOpType
ACTF = mybir.ActivationFunctionType

FP8NP = ml_dtypes.float8_e3m4
BF16NP = ml_dtypes.bfloat16

D = 128
P = 50
N_CORES = 8
B = 16384

# geometry pools: (c slots/sample, s samples/quad, Q quads/group, NG groups/core)
GEOS = ((16, 8, 8, 10), (32, 4, 16, 11), (42, 3, 21, 7), (64, 2, 32, 6))
NG_TOT = sum(g[3] for g in GEOS)  # 34 groups/core
SB = 4  # groups per superblock (one 512-col PSUM bank)
MASKVAL = -30.0

# interleave pools so DMA-heavy c=64 groups spread across the kernel
_items = []
for _gi, (_c, _s, _Q, _NG) in enumerate(GEOS):
    for _k in range(_NG):
        _items.append(((_k + 0.5) / _NG, _gi))
_items.sort()
# GROUPS[g] = (geo_idx, c, s, Q, quad_base)
GROUPS = []
POOL_GROUPS = {gi: [] for gi in range(len(GEOS))}
_qb = 0
for _, _gi in _items:
    _c, _s, _Q, _NG = GEOS[_gi]
    POOL_GROUPS[_gi].append(len(GROUPS))
    GROUPS.append((_gi, _c, _s, _Q, _qb))
    _qb += _Q
NQ_TOT = _qb  # 595 quads/core
NCOL = NG_TOT * 128  # 4352
SLOT_COLS = NQ_TOT * 128  # 76160

# superblocks: (first group, number of groups); ramp up small at the start
# so stage A begins after a small prefetch, ramp down at the tail
_sizes = [1, 1, 2]
while sum(_sizes) + 2 + SB <= NG_TOT:
    _sizes.append(SB)
_rem = NG_TOT - sum(_sizes)
while _rem > 0:
    _sizes.append(min(2, _rem))
    _rem -= min(2, _rem)
SBS = []
_g0 = 0
for _sz in _sizes:
    SBS.append((_g0, _sz))
    _g0 += _sz
assert _g0 == NG_TOT


def build_bass():
    nc = bacc.Bacc(
        "TRN2",
        target_bir_lowering=False,
        debug=False,
        enable_asserts=False,
        num_devices=N_CORES,
    )

    prefT_in = nc.declare_dram_parameter("prefT", [128, SLOT_COLS], FP8, isOutput=False)
    prefQ_in = nc.declare_dram_parameter("prefQ", [128, SLOT_COLS], FP8, isOutput=False)
    tgt_in = nc.declare_dram_parameter("tgt", [128, NCOL], FP8, isOutput=False)
    d0_in = nc.declare_dram_parameter("d0", [128, NCOL], BF16, isOutput=False)
    padc_in = nc.declare_dram_parameter("padc", [1, NCOL], BF16, isOutput=False)
    maskb_in = nc.declare_dram_parameter(
        "maskb", [128, 128 * len(GEOS)], BF16, isOutput=False
    )
    ones8_in = nc.declare_dram_parameter("ones8", [128, 128], FP8, isOutput=False)
    onesr_in = nc.declare_dram_parameter("onesr", [1, 128], BF16, isOutput=False)
    onesc_in = nc.declare_dram_parameter("onesc", [128, 1], BF16, isOutput=False)
    out_d = nc.declare_dram_parameter("out", [1, NCOL], F32, isOutput=True)

    with tile.TileContext(nc) as tc, ExitStack() as ctx:
        ctx.enter_context(
            nc.allow_low_precision(reason="fp8/bf16 pipeline validated vs reference")
        )
        consts = ctx.enter_context(tc.tile_pool(name="consts", bufs=1))
        pT_pool = ctx.enter_context(tc.tile_pool(name="pT", bufs=4))
        pQ_pool = ctx.enter_context(tc.tile_pool(name="pQ", bufs=4))
        tg_pool = ctx.enter_context(tc.tile_pool(name="tg", bufs=4))
        d0_pool = ctx.enter_context(tc.tile_pool(name="d0", bufs=4))
        sm_pool = ctx.enter_context(tc.tile_pool(name="sm", bufs=2))
        q2_pool = ctx.enter_context(tc.tile_pool(name="q2", bufs=2))
        w_ps = ctx.enter_context(
            tc.tile_pool(name="wps", bufs=2, space=bass.MemorySpace.PSUM)
        )
        s_ps = ctx.enter_context(
            tc.tile_pool(name="sps", bufs=2, space=bass.MemorySpace.PSUM)
        )
        r_ps = ctx.enter_context(
            tc.tile_pool(name="rps", bufs=2, space=bass.MemorySpace.PSUM)
        )
        o_ps = ctx.enter_context(
            tc.tile_pool(name="ops", bufs=2, space=bass.MemorySpace.PSUM)
        )

        maskb = consts.tile([128, 128 * len(GEOS)], BF16)
        nc.sync.dma_start(maskb[:], maskb_in[:])
        ones8 = consts.tile([128, 128], FP8)
        nc.sync.dma_start(ones8[:], ones8_in[:])
        onesr = consts.tile([1, 128], BF16)
        nc.sync.dma_start(onesr[:], onesr_in[:])
        onesc = consts.tile([128, 1], BF16)
        nc.sync.dma_start(onesc[:], onesc_in[:])
        padcr = consts.tile([1, NCOL], BF16)
        nc.sync.dma_start(padcr[:], padc_in[:])
        out_s = consts.tile([1, NCOL], F32)

        sb_tiles = [None] * len(SBS)

        def prefetch(sb):
            # spread the big pref reads over the independent DGE paths:
            # pT on the SP HWDGE ring, pQ on the ACT HWDGE ring, small
            # tensors on the GpSimd SWDGE ring.  A single dma_start is
            # already parallelized across all 16 SDMA engines.
            g0, ng = SBS[sb]
            wid = ng * 128
            qb0 = GROUPS[g0][4]
            qb1 = GROUPS[g0 + ng - 1][4] + GROUPS[g0 + ng - 1][3]
            pT = pT_pool.tile([128, (qb1 - qb0) * 128], FP8, tag="pT", name="pT")
            nc.sync.dma_start(pT[:], prefT_in[:, qb0 * 128 : qb1 * 128])
            pQ = pQ_pool.tile([128, (qb1 - qb0) * 128], FP8, tag="pQ", name="pQ")
            nc.scalar.dma_start(pQ[:], prefQ_in[:, qb0 * 128 : qb1 * 128])
            tg = tg_pool.tile([128, wid], FP8, tag="tg", name="tg")
            nc.gpsimd.dma_start(tg[:], tgt_in[:, g0 * 128 : g0 * 128 + wid])
            d0 = d0_pool.tile([128, wid], BF16, tag="d0", name="d0")
            nc.gpsimd.dma_start(d0[:], d0_in[:, g0 * 128 : g0 * 128 + wid])
            sb_tiles[sb] = (pT, pQ, tg, d0, qb0, wid)

        def stage_a(sb):
            g0, ng = SBS[sb]
            pT, pQ, tg, d0, qb0, wid = sb_tiles[sb]
            wps = w_ps.tile([128, 512], F32, tag="w", name="wps")
            for g in range(g0, g0 + ng):
                gi, c, s, Q, qb = GROUPS[g]
                m = 2 * s
                co = (g - g0) * 128
                for q in range(Q):
                    nc.tensor.matmul(
                        wps[:, co + q * m : co + (q + 1) * m],
                        pT[:, (qb - qb0 + q) * 128 : (qb - qb0 + q + 1) * 128],
                        tg[:, co + q * m : co + (q + 1) * m],
                    )
            return wps

        pend_dist = [None]

        def emit_dist():
            if pend_dist[0] is None:
                return
            q2, g0, wid = pend_dist[0]
            pend_dist[0] = None
            ops = o_ps.tile([1, 512], F32, tag="o", name="ops")
            nc.tensor.matmul(ops[:, :wid], onesc[:], q2[:, :wid])
            nc.scalar.copy(out_s[:, g0 * 128 : g0 * 128 + wid], ops[:, :wid])

        def finish(sb, wps):
            g0, ng = SBS[sb]
            pT, pQ, tg, d0, qb0, wid = sb_tiles[sb]
            c0 = g0 * 128

            wm = sm_pool.tile([128, 512], BF16, tag="wm", name="wm")
            for g in range(g0, g0 + ng):
                gi = GROUPS[g][0]
                co = (g - g0) * 128
                nc.vector.tensor_tensor(
                    out=wm[:, co : co + 128],
                    in0=wps[:, co : co + 128],
                    in1=maskb[:, gi * 128 : (gi + 1) * 128],
                    op=ALU.add,
                )
            ee = sm_pool.tile([128, 512], FP8, tag="ee", name="ee")
            nc.scalar.activation(ee[:, :wid], wm[:, :wid], ACTF.Exp)

            sps = s_ps.tile([128, 512], F32, tag="s", name="sps")
            nc.tensor.matmul(
                sps[:, :wid], ones8[:], ee[:, :wid], start=True, stop=False
            )
            nc.tensor.matmul(
                sps[:, :wid],
                onesr[:],
                padcr[:, c0 : c0 + wid],
                start=False,
                stop=True,
                skip_group_check=True,
            )
            rs = sm_pool.tile([128, 512], BF16, tag="rs", name="rs")
            nc.vector.reciprocal(rs[:, :wid], sps[:, :wid])

            rps = r_ps.tile([128, 512], F32, tag="r", name="rps")
            for g in range(g0, g0 + ng):
                gi, c, s, Q, qb = GROUPS[g]
                m = 2 * s
                co = (g - g0) * 128
                for q in range(Q):
                    nc.tensor.matmul(
                        rps[:, co + q * m : co + (q + 1) * m],
                        pQ[:, (qb - qb0 + q) * 128 : (qb - qb0 + q + 1) * 128],
                        ee[:, co + q * m : co + (q + 1) * m],
                    )

            rm = sm_pool.tile([128, 512], BF16, tag="rm", name="rm")
            nc.vector.tensor_tensor(
                out=rm[:, :wid], in0=rps[:, :wid], in1=rs[:, :wid], op=ALU.mult
            )
            qv = sm_pool.tile([128, 512], BF16, tag="qv", name="qv")
            nc.gpsimd.tensor_add(qv[:, :wid], rm[:, :wid], d0[:, :wid])
            q2 = q2_pool.tile([128, 512], BF16, tag="q2", name="q2")
            nc.scalar.activation(q2[:, :wid], qv[:, :wid], ACTF.Square)
            pend_dist[0] = (q2, g0, wid)

        # software pipeline at superblock granularity
        NSB = len(SBS)
        for sb in range(min(3, NSB)):
            prefetch(sb)
        wcur = stage_a(0)
        for sb in range(NSB):
            if sb + 3 < NSB:
                prefetch(sb + 3)
            wnext = stage_a(sb + 1) if sb + 1 < NSB else None
            emit_dist()  # previous superblock's distance reduction
            finish(sb, wcur)
            wcur = wnext
        emit_dist()

        nc.sync.dma_start(out_d[:], out_s[:])

    nc.compile()
    return nc


_CACHE = {}


def _get_bass():
    if "nc" not in _CACHE:
        _CACHE["nc"] = build_bass()
    return _CACHE["nc"]


def _build_masks():
    """Per-geometry block-mask bias [128, 128*len(GEOS)]; dead cols masked."""
    mb = np.full((128, 128 * len(GEOS)), MASKVAL, np.float32)
    rows = np.arange(128)
    for gi, (c, s, Q, NG) in enumerate(GEOS):
        m = 2 * s
        rblk = rows // c  # sample block of each slot row (may exceed s-1)
        cols = np.arange(Q * m)
        cblk = (cols % m) // 2
        blk = rblk[:, None] == cblk[None, :]
        mb[:, gi * 128 : gi * 128 + Q * m] = np.where(blk, 0.0, MASKVAL)
    return np.ascontiguousarray(mb.astype(BF16NP))


_MASKB = _build_masks()


def prep_core(ctx32, ctx8, user_emb, user_ids, pos_ids, neg_ids, pref_ids, v, samples):
    """Build one core's input map.

    samples: dict geo_idx -> int64 array of global sample indices
    Returns (in_map, colsamp, colt, valid) for output unscrambling.
    """
    ZERO = ctx8.shape[0] - 1

    sid = np.full((NQ_TOT, 128), ZERO, np.int64)
    colsamp = np.full(NCOL, -1, np.int64)
    colt = np.zeros(NCOL, np.int64)
    tid = np.full(NCOL, ZERO, np.int64)
    uid = np.zeros(NCOL, np.int64)
    valid = np.zeros(NCOL, bool)
    padc = np.zeros(NCOL, np.float32)

    # dead cols (beyond Q*m within each group's 128): S_raw = 0 -> force S = 1
    for g, (gi, c, s, Q, qb) in enumerate(GROUPS):
        if Q * 2 * s < 128:
            padc[g * 128 + Q * 2 * s : (g + 1) * 128] = 1.0

    for gi, (c, s, Q, NG) in enumerate(GEOS):
        samp = samples[gi]
        n = samp.shape[0]
        spg = Q * s
        assert n <= NG * spg
        r = np.arange(n)
        g_abs = np.asarray(POOL_GROUPS[gi])[r // spg]
        q = (r % spg) // s
        k = (r % spg) % s
        quad = np.array([GROUPS[g][4] for g in g_abs]) + q
        rowbase = k * c

        cp = min(c, P)
        ids = np.full((n, c), ZERO, np.int64)
        ids[:, :cp] = pref_ids[samp, :cp]
        jj = np.arange(c)[None, :]
        ids[jj >= v[samp][:, None]] = ZERO
        sid[quad[:, None], rowbase[:, None] + jj] = ids

        colbase = g_abs * 128 + q * (2 * s) + 2 * k
        for t, t_ids in ((0, pos_ids), (1, neg_ids)):
            cc = colbase + t
            colsamp[cc] = samp
            colt[cc] = t
            tid[cc] = t_ids[samp]
            uid[cc] = user_ids[samp]
            valid[cc] = True
            padc[cc] = -(c - v[samp])

    # pref tensors: gather once, emit both layouts
    g8 = ctx8[sid]  # [NQ_TOT, 128, 128] fp8
    prefQ = np.ascontiguousarray(g8.transpose(1, 0, 2)).reshape(128, SLOT_COLS)
    prefT = np.ascontiguousarray(g8.transpose(2, 0, 1)).reshape(128, SLOT_COLS)

    tgt = np.ascontiguousarray(ctx8[tid].T)  # [128, NCOL] fp8
    d0f = user_emb[uid] - ctx32[tid]  # [NCOL, 128] f32
    d0f[~valid] = 0.0
    d0T = np.ascontiguousarray(d0f.T).astype(BF16NP)

    in_map = {
        "prefT": prefT,
        "prefQ": prefQ,
        "tgt": tgt,
        "d0": d0T,
        "padc": padc.astype(BF16NP).reshape(1, NCOL),
        "maskb": _MASKB,
        "ones8": np.ones((128, 128), FP8NP),
        "onesr": np.ones((1, 128), BF16NP),
        "onesc": np.ones((128, 1), BF16NP),
    }
    return in_map, colsamp, colt, valid


def kernel(user_emb, item_emb, user_ids, pos_ids, neg_ids, pref_ids, n_prefs,
           _trace=False):
    user_emb = np.ascontiguousarray(np.asarray(user_emb, np.float32))
    item_emb = np.asarray(item_emb, np.float32)
    user_ids = np.asarray(user_ids).astype(np.int64)
    pos_ids = np.asarray(pos_ids).astype(np.int64)
    neg_ids = np.asarray(neg_ids).astype(np.int64)
    pref_ids = np.asarray(pref_ids).astype(np.int64)
    n_prefs = np.asarray(n_prefs, np.float32)

    ctx32 = np.concatenate([item_emb, np.zeros((1, D), np.float32)], axis=0)
    ctx8 = ctx32.astype(FP8NP)

    v = n_prefs.astype(np.int64) + 1  # valid slot counts

    # pool assignment with spill-up (smallest feasible c first)
    nb = user_ids.shape[0]
    rem = np.arange(nb)
    pool_of = {}
    for gi, (c, s, Q, NG) in enumerate(GEOS):
        cap = NG * Q * s * N_CORES
        elig = rem[v[rem] <= c]
        chosen = elig[:cap]
        pool_of[gi] = chosen
        rem = np.setdiff1d(rem, chosen, assume_unique=True)
    assert rem.size == 0, "geometry pool capacity overflow"

    nc = _get_bass()

    in_maps = []
    unscr = []
    for core in range(N_CORES):
        samples = {
            gi: np.array_split(pool_of[gi], N_CORES)[core] for gi in range(len(GEOS))
        }
        im, colsamp, colt, valid = prep_core(
            ctx32, ctx8, user_emb, user_ids, pos_ids, neg_ids, pref_ids, v, samples
        )
        in_maps.append(im)
        unscr.append((colsamp, colt, valid))

    res = run_bass_kernel_spmd(
        nc, in_maps, core_ids=list(range(N_CORES)), trace=_trace
    )

    out = np.empty((2, nb), dtype=np.float32)
    for core in range(N_CORES):
        r = np.asarray(res.results[core]["out"]).reshape(NCOL)
        colsamp, colt, valid = unscr[core]
        out[colt[valid], colsamp[valid]] = r[valid]
    if _trace:
        return out, res
    return out
